# revision 2
# baseline (speedup 1.0000x reference)
"""Trainium2 Bass kernel for nn_CapsuleSequenceToGraph.

Strategy (8 NeuronCores, single SPMD NEFF):
  - Shard the sequence dim T across cores (weights are the dominant HBM
    traffic; T-sharding reads each weight byte exactly once chip-wide).
  - Per core: pri = einsum('btj,tnjd->btnd') via PE matmuls, two t's packed
    per matmul with a block-diagonal x as the stationary operand.
    pri kept in SBUF as bf16, tile layout [part=(t2,b=64), free=(d,n)].
  - Dynamic routing (3 rounds + final readout):
      s_r = sum_t softmax_n(b_r) * pri   -> cross-core AllReduce of [B, n*d]
      v_r = tanh(s_r);  V_r = sum v_r    (running sum)
      b_{r+1} = sum_d V_r * pri          (fresh each round; b_0 = 0)
    Round 0's softmax over zeros is uniform, so s_0 = sum_t pri / 32 is
    accumulated directly on the PE while pri is being produced.
    The t/tile reduction of s runs on the PE via a stacked-identity selector
    with PSUM accumulation.  Elementwise muls run on the DVE in bf16.
    exp/tanh on the scalar engine.  |b| < 0.02 for these inputs so softmax
    needs no max-subtraction.
  - The final s_3 is NOT allreduced: each core emits its partial sum and the
    host reduces + applies tanh (saves one collective round per modality).
"""

import sys

if "/opt/trn_rl_repo" not in sys.path:
    sys.path.insert(0, "/opt/trn_rl_repo")

import numpy as np
import ml_dtypes

import concourse.bass as bass
import concourse.bacc as bacc
import concourse.mybir as mybir
from concourse import tile
from concourse.bass_utils import run_bass_kernel_spmd

F32 = mybir.dt.float32
BF16 = mybir.dt.bfloat16
AF = mybir.ActivationFunctionType
ALU = mybir.AluOpType

N_CORES = 8
B = 64
NV = 32  # n vertices
DC = 16  # capsule dim
J = 64  # MULT_D
T_DIMS = {"text": 128, "audio": 512, "video": 256, "frames": 256}
W_NAMES = {"text": "W_tpc", "audio": "W_apc", "video": "W_vpc", "frames": "W_fpc"}
# emit order: small modalities first so their collectives fire while the
# big ones are still streaming weights
ORDER = ["text", "video", "frames", "audio"]
OUT_ORDER = ["text", "audio", "video", "frames"]
ROUNDS = 3
FN = DC * NV  # 512, free dim (d-major: flat = d*32 + n)

_CACHE = {}


def _pairs(mod):
    return T_DIMS[mod] // N_CORES // 2


def _build():
    nc = bacc.Bacc("TRN2", target_bir_lowering=False, debug=False, num_devices=N_CORES)

    xb_d = {}
    wr_d = {}
    out_d = {}
    for mod in ORDER:
        P = _pairs(mod)
        xb_d[mod] = nc.dram_tensor(f"xb_{mod}", [P, 128, 128], F32, kind="ExternalInput")
        wr_d[mod] = nc.dram_tensor(f"wr_{mod}", [P, 128, FN], F32, kind="ExternalInput")
        out_d[mod] = nc.dram_tensor(f"out_{mod}", [B, FN], F32, kind="ExternalOutput")
    sel_d = nc.dram_tensor("sel", [128, 64], BF16, kind="ExternalInput")

    rg = [list(range(N_CORES))]

    with tile.TileContext(nc) as tc:
        with (
            tc.tile_pool(name="io", bufs=3) as io,
            tc.tile_pool(name="pri", bufs=1) as pri_pool,
            tc.tile_pool(name="state", bufs=1) as st,
            tc.tile_pool(name="wk", bufs=4) as wk,
            tc.tile_pool(name="sm", bufs=2) as sm,
            tc.tile_pool(name="pp", bufs=2, space="PSUM") as ps_pri,
            tc.tile_pool(name="psacc", bufs=1, space="PSUM") as ps_s,
            tc.tile_pool(name="dram", bufs=1, space="DRAM") as dr,
        ):
            sel = st.tile([128, 64], BF16, tag="sel", name="sel")
            nc.sync.dma_start(sel[:], sel_d[:])

            pri = {}  # mod -> list of [128, FN] bf16 tiles
            vvbf = {}  # mod -> [128, FN] bf16 (V duplicated in both halves)
            Vf = {}  # mod -> [64, FN] f32 running sum of tanh
            bstate = {}  # mod -> [128, P*NV] f32
            estate = {}  # mod -> [128, P*NV] bf16
            den = {}
            rinv = {}
            s_glob = {}  # mod -> [64, FN] f32 allreduced s of current round

            def alloc_state(mod):
                P = _pairs(mod)
                vvbf[mod] = st.tile([128, FN], BF16, tag=f"vv_{mod}", name=f"vv_{mod}")
                Vf[mod] = st.tile([64, FN], F32, tag=f"V_{mod}", name=f"V_{mod}")
                bstate[mod] = st.tile([128, P * NV], F32, tag=f"b_{mod}", name=f"b_{mod}")
                estate[mod] = st.tile([128, P * NV], BF16, tag=f"e_{mod}", name=f"e_{mod}")
                den[mod] = st.tile([128, P], F32, tag=f"den_{mod}", name=f"den_{mod}")
                rinv[mod] = st.tile([128, P], F32, tag=f"ri_{mod}", name=f"ri_{mod}")

            def emit_allreduce(mod, r, s_psum):
                """PSUM [64,FN] partial -> DRAM -> AllReduce -> SBUF f32."""
                s_loc = sm.tile([64, FN], F32, tag=f"sl_{mod}", name=f"sl_{mod}")
                nc.scalar.copy(s_loc[:], s_psum[:])
                ar_in = dr.tile([64, FN], F32, tag=f"ari_{mod}_{r}", name=f"ari_{mod}_{r}")
                ar_out = dr.tile([64, FN], F32, tag=f"aro_{mod}_{r}", name=f"aro_{mod}_{r}")
                nc.gpsimd.dma_start(ar_in[:], s_loc[:])
                nc.gpsimd.collective_compute(
                    "AllReduce",
                    ALU.add,
                    replica_groups=rg,
                    ins=[ar_in.opt()],
                    outs=[ar_out.opt()],
                )
                sg = sm.tile([64, FN], F32, tag=f"sg_{mod}", name=f"sg_{mod}")
                nc.gpsimd.dma_start(sg[:], ar_out[:])
                s_glob[mod] = sg

            # ---------- phase 1: pri + s0 accumulation ----------
            for mod in ORDER:
                P = _pairs(mod)
                alloc_state(mod)
                pri[mod] = []
                s_ps = ps_s.tile([64, FN], F32, tag=f"s_{mod}", name=f"s_{mod}")
                for p in range(P):
                    xb_t = io.tile([128, 128], F32, tag="xb", name="xb_t")
                    nc.sync.dma_start(xb_t[:], xb_d[mod][p])
                    wr_t = io.tile([128, FN], F32, tag="wr", name="wr_t")
                    nc.sync.dma_start(wr_t[:], wr_d[mod][p])
                    pp = ps_pri.tile([128, FN], F32, tag="pp", name="pp")
                    nc.tensor.matmul(pp[:], xb_t[:], wr_t[:], start=True, stop=True)
                    pri_t = pri_pool.tile([128, FN], BF16, tag=f"pri_{mod}_{p}", name=f"pri_{mod}_{p}")
                    nc.scalar.copy(pri_t[:], pp[:])
                    pri[mod].append(pri_t)
                    nc.tensor.matmul(
                        s_ps[:],
                        sel[:],
                        pri_t[:],
                        start=(p == 0),
                        stop=(p == P - 1),
                    )
                emit_allreduce(mod, 0, s_ps)

            # ---------- phase 2: routing rounds ----------
            def v_update_and_b(mod, r):
                """tanh(s_r) -> V; w = pri*V; b = sum_d w; softmax prep."""
                P = _pairs(mod)
                t_tmp = sm.tile([64, FN], F32, tag=f"vt_{mod}", name=f"vt_{mod}")
                scale = (1.0 / NV) if r == 0 else 1.0
                nc.scalar.activation(t_tmp[:], s_glob[mod][:], AF.Tanh, scale=scale)
                if r == 0:
                    nc.vector.tensor_copy(Vf[mod][:], t_tmp[:])
                else:
                    nc.vector.tensor_tensor(
                        out=Vf[mod][:], in0=Vf[mod][:], in1=t_tmp[:], op=ALU.add
                    )
                nc.vector.tensor_copy(vvbf[mod][0:64, :], Vf[mod][:])
                nc.vector.tensor_copy(vvbf[mod][64:128, :], Vf[mod][:])
                for p in range(P):
                    w_t = wk.tile([128, FN], BF16, tag="w", name="w_t")
                    nc.vector.tensor_tensor(
                        out=w_t[:], in0=pri[mod][p][:], in1=vvbf[mod][:], op=ALU.mult
                    )
                    nc.vector.tensor_reduce(
                        out=bstate[mod][:, p * NV : (p + 1) * NV],
                        in_=w_t.rearrange("q (d n) -> q n d", d=DC),
                        axis=mybir.AxisListType.X,
                        op=ALU.add,
                    )
                nc.scalar.activation(estate[mod][:], bstate[mod][:], AF.Exp)
                nc.vector.tensor_reduce(
                    out=den[mod][:],
                    in_=estate[mod].rearrange("q (t n) -> q t n", n=NV),
                    axis=mybir.AxisListType.X,
                    op=ALU.add,
                )
                nc.vector.reciprocal(rinv[mod][:], den[mod][:])

            def mul1_and_s(mod, r):
                """m = rc * pri ; s_psum = sum_t m via selector matmuls."""
                P = _pairs(mod)
                s_ps = ps_s.tile([64, FN], F32, tag=f"s_{mod}", name=f"s_{mod}")
                for p in range(P):
                    m_t = wk.tile([128, FN], BF16, tag="m", name="m_t")
                    nc.vector.scalar_tensor_tensor(
                        out=m_t.rearrange("q (d n) -> q d n", d=DC),
                        in0=pri[mod][p].rearrange("q (d n) -> q d n", d=DC),
                        scalar=rinv[mod][:, p : p + 1],
                        in1=estate[mod][:, p * NV : (p + 1) * NV]
                        .unsqueeze(1)
                        .broadcast_to([128, DC, NV]),
                        op0=ALU.mult,
                        op1=ALU.mult,
                    )
                    nc.tensor.matmul(
                        s_ps[:],
                        sel[:],
                        m_t[:],
                        start=(p == 0),
                        stop=(p == P - 1),
                    )
                return s_ps

            for r in range(ROUNDS):
                for mod in ORDER:
                    v_update_and_b(mod, r)
                    s_ps = mul1_and_s(mod, r + 1)
                    if r < ROUNDS - 1:
                        emit_allreduce(mod, r + 1, s_ps)
                    else:
                        s_out = sm.tile([64, FN], F32, tag=f"so_{mod}", name=f"so_{mod}")
                        nc.scalar.copy(s_out[:], s_ps[:])
                        nc.sync.dma_start(out_d[mod][:], s_out[:])

    nc.compile()
    return nc


def _host_prep(inputs):
    """Build the 8 per-core input maps (T-sharded, PE-ready layouts)."""
    sel = np.concatenate([np.eye(64, dtype=np.float32)] * 2, axis=0).astype(
        ml_dtypes.bfloat16
    )
    in_maps = []
    for c in range(N_CORES):
        m = {"sel": sel}
        for mod in ORDER:
            T = T_DIMS[mod]
            Tc = T // N_CORES
            P = Tc // 2
            t0 = c * Tc
            x = np.asarray(inputs[mod], dtype=np.float32)  # [B, T, J]
            W = np.asarray(inputs[W_NAMES[mod]], dtype=np.float32)  # [T,NV,J,DC]
            xs = np.ascontiguousarray(
                x[:, t0 : t0 + Tc, :].transpose(1, 2, 0)
            )  # [Tc, J, B]
            xb = np.zeros((P, 128, 128), dtype=np.float32)
            xb[:, 0:64, 0:64] = xs[0::2]
            xb[:, 64:128, 64:128] = xs[1::2]
            wt = W[t0 : t0 + Tc].transpose(0, 2, 3, 1).reshape(Tc, J, FN)
            # wt[t, j, d*32+n] = W[t, n, j, d]
            wr = np.empty((P, 128, FN), dtype=np.float32)
            wr[:, 0:64, :] = wt[0::2]
            wr[:, 64:128, :] = wt[1::2]
            m[f"xb_{mod}"] = xb
            m[f"wr_{mod}"] = np.ascontiguousarray(wr)
        in_maps.append(m)
    return in_maps


def _gather(results):
    outs = []
    for mod in OUT_ORDER:
        s = np.zeros((B, FN), dtype=np.float64)
        for c in range(N_CORES):
            s += np.asarray(results[c][f"out_{mod}"], dtype=np.float64)
        o = np.tanh(s.astype(np.float32))
        outs.append(np.ascontiguousarray(o.reshape(B, DC, NV).transpose(0, 2, 1)))
    return tuple(outs)


def kernel(**inputs):
    if "nc" not in _CACHE:
        _CACHE["nc"] = _build()
    nc = _CACHE["nc"]
    in_maps = _host_prep(inputs)
    res = run_bass_kernel_spmd(nc, in_maps, core_ids=list(range(N_CORES)))
    return _gather(res.results)


# revision 6
# speedup vs baseline: 1.1740x; 1.1740x over previous
"""Trainium2 Bass kernel for nn_CapsuleSequenceToGraph.

Strategy (8 NeuronCores, single SPMD NEFF):
  - Shard the sequence dim T across cores (weights are the dominant HBM
    traffic; T-sharding reads each weight byte exactly once chip-wide).
  - Per core: pri = einsum('btj,tnjd->btnd') via PE matmuls, two t's packed
    per matmul with a block-diagonal x as the stationary operand.
    pri kept in SBUF as bf16, tile layout [part=(t2,b=64), free=(d,n)].
  - Dynamic routing (3 rounds + final readout):
      s_r = sum_t softmax_n(b_r) * pri   -> cross-core AllReduce of [B, n*d]
      v_r = tanh(s_r);  V_r = sum v_r    (running sum)
      b_{r+1} = sum_d V_r * pri          (fresh each round; b_0 = 0)
    Round 0's softmax over zeros is uniform, so s_0 = sum_t pri / 32 is
    accumulated directly on the PE while pri is being produced.
    The t/tile reduction of s runs on the PE via a stacked-identity selector
    with PSUM accumulation.  Elementwise muls run on the DVE in bf16.
    exp/tanh on the scalar engine.  |b| < 0.02 for these inputs so softmax
    needs no max-subtraction.
  - The final s_3 is NOT allreduced: each core emits its partial sum and the
    host reduces + applies tanh (saves one collective round per modality).
"""

import sys

if "/opt/trn_rl_repo" not in sys.path:
    sys.path.insert(0, "/opt/trn_rl_repo")

import numpy as np
import ml_dtypes

import concourse.bass as bass
import concourse.bacc as bacc
import concourse.mybir as mybir
from concourse import tile
from concourse.bass_utils import run_bass_kernel_spmd

F32 = mybir.dt.float32
BF16 = mybir.dt.bfloat16
AF = mybir.ActivationFunctionType
ALU = mybir.AluOpType

N_CORES = 8
B = 64
NV = 32  # n vertices
DC = 16  # capsule dim
J = 64  # MULT_D
T_DIMS = {"text": 128, "audio": 512, "video": 256, "frames": 256}
W_NAMES = {"text": "W_tpc", "audio": "W_apc", "video": "W_vpc", "frames": "W_fpc"}
# emit order: small modalities first so their collectives fire while the
# big ones are still streaming weights
ORDER = ["text", "video", "frames", "audio"]
OUT_ORDER = ["text", "audio", "video", "frames"]
ROUNDS = 3
FN = DC * NV  # 512, free dim (d-major: flat = d*32 + n)

_CACHE = {}


def _pairs(mod):
    return T_DIMS[mod] // N_CORES // 2


def _build():
    nc = bacc.Bacc("TRN2", target_bir_lowering=False, debug=False, num_devices=N_CORES)

    xb_d = {}
    wr_d = {}
    out_d = {}
    for mod in ORDER:
        P = _pairs(mod)
        xb_d[mod] = nc.dram_tensor(f"xb_{mod}", [P, 128, 128], F32, kind="ExternalInput")
        wr_d[mod] = nc.dram_tensor(f"wr_{mod}", [P, 128, FN], F32, kind="ExternalInput")
        out_d[mod] = nc.dram_tensor(f"out_{mod}", [B, FN], F32, kind="ExternalOutput")
    sel_d = nc.dram_tensor("sel", [128, 64], BF16, kind="ExternalInput")

    rg = [list(range(N_CORES))]

    with tile.TileContext(nc) as tc:
        with (
            tc.tile_pool(name="io", bufs=3) as io,
            tc.tile_pool(name="pri", bufs=1) as pri_pool,
            tc.tile_pool(name="state", bufs=1) as st,
            tc.tile_pool(name="wk", bufs=4) as wk,
            tc.tile_pool(name="sm", bufs=2) as sm,
            tc.tile_pool(name="pp", bufs=2, space="PSUM") as ps_pri,
            tc.tile_pool(name="psacc", bufs=1, space="PSUM") as ps_s,
            tc.tile_pool(name="dram", bufs=1, space="DRAM") as dr,
        ):
            sel = st.tile([128, 64], BF16, tag="sel", name="sel")
            nc.sync.dma_start(sel[:], sel_d[:])

            pri = {}  # mod -> list of [128, FN] bf16 tiles
            vvbf = {}  # mod -> [128, FN] bf16 (V duplicated in both halves)
            Vf = {}  # mod -> [64, FN] f32 running sum of tanh
            bstate = {}  # mod -> [128, P*NV] f32
            estate = {}  # mod -> [128, P*NV] bf16
            den = {}
            rinv = {}
            rcbf = {}
            s_glob = {}  # mod -> [64, FN] f32 allreduced s of current round

            def alloc_state(mod):
                P = _pairs(mod)
                vvbf[mod] = st.tile([128, FN], BF16, tag=f"vv_{mod}", name=f"vv_{mod}")
                Vf[mod] = st.tile([64, FN], F32, tag=f"V_{mod}", name=f"V_{mod}")
                bstate[mod] = st.tile([128, P * NV], F32, tag=f"b_{mod}", name=f"b_{mod}")
                estate[mod] = st.tile([128, P * NV], BF16, tag=f"e_{mod}", name=f"e_{mod}")
                den[mod] = st.tile([128, P], F32, tag=f"den_{mod}", name=f"den_{mod}")
                rinv[mod] = st.tile([128, P], F32, tag=f"ri_{mod}", name=f"ri_{mod}")
                rcbf[mod] = st.tile([128, P * NV], BF16, tag=f"rc_{mod}", name=f"rc_{mod}")

            def emit_allreduce(mod, r, s_psum):
                """PSUM [64,FN] partial -> DRAM -> AllReduce -> SBUF f32."""
                s_loc = sm.tile([64, FN], F32, tag=f"sl_{mod}", name=f"sl_{mod}")
                nc.scalar.copy(s_loc[:], s_psum[:])
                ar_in = dr.tile([64, FN], F32, tag=f"ari_{mod}_{r}", name=f"ari_{mod}_{r}")
                ar_out = dr.tile([64, FN], F32, tag=f"aro_{mod}_{r}", name=f"aro_{mod}_{r}")
                nc.sync.dma_start(ar_in[:], s_loc[:])
                nc.gpsimd.collective_compute(
                    "AllReduce",
                    ALU.add,
                    replica_groups=rg,
                    ins=[ar_in.opt()],
                    outs=[ar_out.opt()],
                )
                sg = sm.tile([64, FN], F32, tag=f"sg_{mod}", name=f"sg_{mod}")
                nc.sync.dma_start(sg[:], ar_out[:])
                s_glob[mod] = sg

            # ---------- phase 1: pri + s0 accumulation ----------
            for mod in ORDER:
                P = _pairs(mod)
                alloc_state(mod)
                pri[mod] = []
                s_ps = ps_s.tile([64, FN], F32, tag=f"s_{mod}", name=f"s_{mod}")
                for p in range(P):
                    xb_t = io.tile([128, 128], F32, tag="xb", name="xb_t")
                    nc.sync.dma_start(xb_t[:], xb_d[mod][p])
                    wr_t = io.tile([128, FN], F32, tag="wr", name="wr_t")
                    nc.sync.dma_start(wr_t[:], wr_d[mod][p])
                    pp = ps_pri.tile([128, FN], F32, tag="pp", name="pp")
                    nc.tensor.matmul(pp[:], xb_t[:], wr_t[:], start=True, stop=True)
                    pri_t = pri_pool.tile([128, FN], BF16, tag=f"pri_{mod}_{p}", name=f"pri_{mod}_{p}")
                    nc.scalar.copy(pri_t[:], pp[:])
                    pri[mod].append(pri_t)
                    nc.tensor.matmul(
                        s_ps[:],
                        sel[:],
                        pri_t[:],
                        start=(p == 0),
                        stop=(p == P - 1),
                    )
                emit_allreduce(mod, 0, s_ps)

            # ---------- phase 2: routing rounds ----------
            def v_update_and_b(mod, r):
                """tanh(s_r) -> V; w = pri*V; b = sum_d w; softmax prep."""
                P = _pairs(mod)
                t_tmp = sm.tile([64, FN], F32, tag=f"vt_{mod}", name=f"vt_{mod}")
                scale = (1.0 / NV) if r == 0 else 1.0
                nc.scalar.activation(t_tmp[:], s_glob[mod][:], AF.Tanh, scale=scale)
                if r == 0:
                    nc.vector.tensor_copy(Vf[mod][:], t_tmp[:])
                else:
                    nc.vector.tensor_tensor(
                        out=Vf[mod][:], in0=Vf[mod][:], in1=t_tmp[:], op=ALU.add
                    )
                nc.vector.tensor_copy(vvbf[mod][0:64, :], Vf[mod][:])
                nc.vector.tensor_copy(vvbf[mod][64:128, :], Vf[mod][:])
                for p in range(P):
                    w_t = wk.tile([128, FN], BF16, tag="w", name="w_t")
                    nc.vector.tensor_tensor(
                        out=w_t[:], in0=pri[mod][p][:], in1=vvbf[mod][:], op=ALU.mult
                    )
                    nc.vector.tensor_reduce(
                        out=bstate[mod][:, p * NV : (p + 1) * NV],
                        in_=w_t.rearrange("q (d n) -> q n d", d=DC),
                        axis=mybir.AxisListType.X,
                        op=ALU.add,
                    )
                nc.scalar.activation(estate[mod][:], bstate[mod][:], AF.Exp)
                nc.vector.tensor_reduce(
                    out=den[mod][:],
                    in_=estate[mod].rearrange("q (t n) -> q t n", n=NV),
                    axis=mybir.AxisListType.X,
                    op=ALU.add,
                )
                nc.vector.reciprocal(rinv[mod][:], den[mod][:])
                nc.vector.tensor_tensor(
                    out=rcbf[mod].rearrange("q (t n) -> q t n", n=NV),
                    in0=estate[mod].rearrange("q (t n) -> q t n", n=NV),
                    in1=rinv[mod].unsqueeze(2).broadcast_to([128, P, NV]),
                    op=ALU.mult,
                )

            def mul1_and_s(mod, r):
                """m = rc * pri ; s_psum = sum_t m via selector matmuls."""
                P = _pairs(mod)
                s_ps = ps_s.tile([64, FN], F32, tag=f"s_{mod}", name=f"s_{mod}")
                for p in range(P):
                    m_t = wk.tile([128, FN], BF16, tag="m", name="m_t")
                    nc.vector.tensor_tensor(
                        out=m_t.rearrange("q (d n) -> q d n", d=DC),
                        in0=pri[mod][p].rearrange("q (d n) -> q d n", d=DC),
                        in1=rcbf[mod][:, p * NV : (p + 1) * NV]
                        .unsqueeze(1)
                        .broadcast_to([128, DC, NV]),
                        op=ALU.mult,
                    )
                    nc.tensor.matmul(
                        s_ps[:],
                        sel[:],
                        m_t[:],
                        start=(p == 0),
                        stop=(p == P - 1),
                    )
                return s_ps

            for r in range(ROUNDS):
                for mod in ORDER:
                    v_update_and_b(mod, r)
                    s_ps = mul1_and_s(mod, r + 1)
                    if r < ROUNDS - 1:
                        emit_allreduce(mod, r + 1, s_ps)
                    else:
                        s_out = sm.tile([64, FN], F32, tag=f"so_{mod}", name=f"so_{mod}")
                        nc.scalar.copy(s_out[:], s_ps[:])
                        nc.sync.dma_start(out_d[mod][:], s_out[:])

    nc.compile()
    return nc


def _host_prep(inputs):
    """Build the 8 per-core input maps (T-sharded, PE-ready layouts)."""
    sel = np.concatenate([np.eye(64, dtype=np.float32)] * 2, axis=0).astype(
        ml_dtypes.bfloat16
    )
    in_maps = []
    for c in range(N_CORES):
        m = {"sel": sel}
        for mod in ORDER:
            T = T_DIMS[mod]
            Tc = T // N_CORES
            P = Tc // 2
            t0 = c * Tc
            x = np.asarray(inputs[mod], dtype=np.float32)  # [B, T, J]
            W = np.asarray(inputs[W_NAMES[mod]], dtype=np.float32)  # [T,NV,J,DC]
            xs = np.ascontiguousarray(
                x[:, t0 : t0 + Tc, :].transpose(1, 2, 0)
            )  # [Tc, J, B]
            xb = np.zeros((P, 128, 128), dtype=np.float32)
            xb[:, 0:64, 0:64] = xs[0::2]
            xb[:, 64:128, 64:128] = xs[1::2]
            wt = W[t0 : t0 + Tc].transpose(0, 2, 3, 1).reshape(Tc, J, FN)
            # wt[t, j, d*32+n] = W[t, n, j, d]
            wr = np.empty((P, 128, FN), dtype=np.float32)
            wr[:, 0:64, :] = wt[0::2]
            wr[:, 64:128, :] = wt[1::2]
            m[f"xb_{mod}"] = xb
            m[f"wr_{mod}"] = np.ascontiguousarray(wr)
        in_maps.append(m)
    return in_maps


def _gather(results):
    outs = []
    for mod in OUT_ORDER:
        s = np.zeros((B, FN), dtype=np.float64)
        for c in range(N_CORES):
            s += np.asarray(results[c][f"out_{mod}"], dtype=np.float64)
        o = np.tanh(s.astype(np.float32))
        outs.append(np.ascontiguousarray(o.reshape(B, DC, NV).transpose(0, 2, 1)))
    return tuple(outs)


def kernel(**inputs):
    if "nc" not in _CACHE:
        _CACHE["nc"] = _build()
    nc = _CACHE["nc"]
    in_maps = _host_prep(inputs)
    res = run_bass_kernel_spmd(nc, in_maps, core_ids=list(range(N_CORES)))
    return _gather(res.results)


# revision 8
# speedup vs baseline: 2.0336x; 1.7321x over previous
"""Trainium2 Bass kernel for nn_CapsuleSequenceToGraph.

Strategy (8 NeuronCores, single SPMD NEFF):
  - Shard the sequence dim T across cores (weights are the dominant HBM
    traffic; T-sharding reads each weight byte exactly once chip-wide).
  - Per core: pri = einsum('btj,tnjd->btnd') via PE matmuls, two t's packed
    per matmul with a block-diagonal x as the stationary operand.
    pri kept in SBUF as bf16, tile layout [part=(t2,b=64), free=(d,n)].
  - Dynamic routing (3 rounds + final readout):
      s_r = sum_t softmax_n(b_r) * pri   -> cross-core AllReduce of [B, n*d]
      v_r = tanh(s_r);  V_r = sum v_r    (running sum)
      b_{r+1} = sum_d V_r * pri          (fresh each round; b_0 = 0)
    Round 0's softmax over zeros is uniform, so s_0 = sum_t pri / 32 is
    accumulated directly on the PE while pri is being produced.
    The t/tile reduction of s runs on the PE via a stacked-identity selector
    with PSUM accumulation.  Elementwise muls run on the DVE in bf16.
    exp/tanh on the scalar engine.  |b| < 0.02 for these inputs so softmax
    needs no max-subtraction.
  - The final s_3 is NOT allreduced: each core emits its partial sum and the
    host reduces + applies tanh (saves one collective round per modality).
"""

import sys

if "/opt/trn_rl_repo" not in sys.path:
    sys.path.insert(0, "/opt/trn_rl_repo")

import numpy as np
import ml_dtypes

import concourse.bass as bass
import concourse.bacc as bacc
import concourse.mybir as mybir
from concourse import tile
from concourse.bass_utils import run_bass_kernel_spmd

F32 = mybir.dt.float32
BF16 = mybir.dt.bfloat16
AF = mybir.ActivationFunctionType
ALU = mybir.AluOpType

N_CORES = 8
B = 64
NV = 32  # n vertices
DC = 16  # capsule dim
J = 64  # MULT_D
T_DIMS = {"text": 128, "audio": 512, "video": 256, "frames": 256}
W_NAMES = {"text": "W_tpc", "audio": "W_apc", "video": "W_vpc", "frames": "W_fpc"}
# emit order: small modalities first so their collectives fire while the
# big ones are still streaming weights
ORDER = ["text", "video", "frames", "audio"]
OUT_ORDER = ["text", "audio", "video", "frames"]
ROUNDS = 3
FN = DC * NV  # 512, free dim (d-major: flat = d*32 + n)

_CACHE = {}
AR_MODE = "cc"  # "cc" = real AllReduce; "copy" = local bounce only (timing expt)


def _pairs(mod):
    return T_DIMS[mod] // N_CORES // 2


def _build():
    nc = bacc.Bacc("TRN2", target_bir_lowering=False, debug=False, num_devices=N_CORES)

    xb_d = {}
    wr_d = {}
    out_d = {}
    for mod in ORDER:
        P = _pairs(mod)
        xb_d[mod] = nc.dram_tensor(f"xb_{mod}", [P, 128, 128], F32, kind="ExternalInput")
        wr_d[mod] = nc.dram_tensor(f"wr_{mod}", [P, 128, FN], F32, kind="ExternalInput")
        out_d[mod] = nc.dram_tensor(f"out_{mod}", [B, FN], F32, kind="ExternalOutput")
    sel_d = nc.dram_tensor("sel", [128, 64], BF16, kind="ExternalInput")

    rg = [list(range(N_CORES))]

    with tile.TileContext(nc) as tc:
        with (
            tc.tile_pool(name="io", bufs=3) as io,
            tc.tile_pool(name="pri", bufs=1) as pri_pool,
            tc.tile_pool(name="state", bufs=1) as st,
            tc.tile_pool(name="wk", bufs=4) as wk,
            tc.tile_pool(name="sm", bufs=2) as sm,
            tc.tile_pool(name="pp", bufs=2, space="PSUM") as ps_pri,
            tc.tile_pool(name="psacc", bufs=1, space="PSUM") as ps_s,
            tc.tile_pool(name="dram", bufs=1, space="DRAM") as dr,
        ):
            sel = st.tile([128, 64], BF16, tag="sel", name="sel")
            nc.sync.dma_start(sel[:], sel_d[:])

            pri = {}  # mod -> list of [128, FN] bf16 tiles
            vvbf = {}  # mod -> [128, FN] bf16 (V duplicated in both halves)
            Vf = {}  # mod -> [64, FN] f32 running sum of tanh
            bstate = {}  # mod -> [128, P*NV] f32
            estate = {}  # mod -> [128, P*NV] bf16
            den = {}
            rinv = {}
            rcbf = {}
            s_glob = {}  # mod -> [64, FN] f32 allreduced s of current round
            arb = {}  # round -> (ar_in, ar_out) [256, FN] batched buffers

            def alloc_state(mod):
                P = _pairs(mod)
                vvbf[mod] = st.tile([128, FN], BF16, tag=f"vv_{mod}", name=f"vv_{mod}")
                Vf[mod] = st.tile([64, FN], F32, tag=f"V_{mod}", name=f"V_{mod}")
                bstate[mod] = st.tile([128, P * NV], F32, tag=f"b_{mod}", name=f"b_{mod}")
                estate[mod] = st.tile([128, P * NV], BF16, tag=f"e_{mod}", name=f"e_{mod}")
                den[mod] = st.tile([128, P], F32, tag=f"den_{mod}", name=f"den_{mod}")
                rinv[mod] = st.tile([128, P], F32, tag=f"ri_{mod}", name=f"ri_{mod}")
                rcbf[mod] = st.tile([128, P * NV], BF16, tag=f"rc_{mod}", name=f"rc_{mod}")

            def emit_allreduce(mod, r, s_psum):
                """PSUM [64,FN] partial -> DRAM -> AllReduce -> SBUF f32."""
                if AR_MODE == "batch":
                    mi = ORDER.index(mod)
                    if r not in arb:
                        bi = dr.tile([4 * 64, FN], F32, tag=f"abi_{r}", name=f"abi_{r}")
                        bo = dr.tile([4 * 64, FN], F32, tag=f"abo_{r}", name=f"abo_{r}")
                        arb[r] = (bi, bo)
                    bi, bo = arb[r]
                    s_loc = sm.tile([64, FN], F32, tag=f"sl_{mod}", name=f"sl_{mod}")
                    nc.scalar.copy(s_loc[:], s_psum[:])
                    nc.sync.dma_start(bi[mi * 64 : (mi + 1) * 64, :], s_loc[:])
                    if mi == len(ORDER) - 1:
                        nc.gpsimd.collective_compute(
                            "AllReduce",
                            ALU.add,
                            replica_groups=rg,
                            ins=[bi.opt()],
                            outs=[bo.opt()],
                        )
                        for mod2 in ORDER:
                            mj = ORDER.index(mod2)
                            sg = sm.tile([64, FN], F32, tag=f"sg_{mod2}", name=f"sg_{mod2}")
                            nc.sync.dma_start(sg[:], bo[mj * 64 : (mj + 1) * 64, :])
                            s_glob[mod2] = sg
                    return
                s_loc = sm.tile([64, FN], F32, tag=f"sl_{mod}", name=f"sl_{mod}")
                nc.scalar.copy(s_loc[:], s_psum[:])
                ar_in = dr.tile([64, FN], F32, tag=f"ari_{mod}_{r}", name=f"ari_{mod}_{r}")
                ar_out = dr.tile([64, FN], F32, tag=f"aro_{mod}_{r}", name=f"aro_{mod}_{r}")
                nc.sync.dma_start(ar_in[:], s_loc[:])
                if AR_MODE == "cc":
                    nc.gpsimd.collective_compute(
                        "AllReduce",
                        ALU.add,
                        replica_groups=rg,
                        ins=[ar_in.opt()],
                        outs=[ar_out.opt()],
                    )
                else:
                    nc.sync.dma_start(ar_out[:], ar_in[:])
                sg = sm.tile([64, FN], F32, tag=f"sg_{mod}", name=f"sg_{mod}")
                nc.sync.dma_start(sg[:], ar_out[:])
                s_glob[mod] = sg

            # ---------- phase 1: pri + s0 accumulation ----------
            for mod in ORDER:
                P = _pairs(mod)
                alloc_state(mod)
                pri[mod] = []
                s_ps = ps_s.tile([64, FN], F32, tag=f"s_{mod}", name=f"s_{mod}")
                for p in range(P):
                    xb_t = io.tile([128, 128], F32, tag="xb", name="xb_t")
                    nc.sync.dma_start(xb_t[:], xb_d[mod][p])
                    wr_t = io.tile([128, FN], F32, tag="wr", name="wr_t")
                    nc.sync.dma_start(wr_t[:], wr_d[mod][p])
                    pp = ps_pri.tile([128, FN], F32, tag="pp", name="pp")
                    nc.tensor.matmul(pp[:], xb_t[:], wr_t[:], start=True, stop=True)
                    pri_t = pri_pool.tile([128, FN], BF16, tag=f"pri_{mod}_{p}", name=f"pri_{mod}_{p}")
                    nc.scalar.copy(pri_t[:], pp[:])
                    pri[mod].append(pri_t)
                    nc.tensor.matmul(
                        s_ps[:],
                        sel[:],
                        pri_t[:],
                        start=(p == 0),
                        stop=(p == P - 1),
                    )
                emit_allreduce(mod, 0, s_ps)

            # ---------- phase 2: routing rounds ----------
            def v_update_and_b(mod, r):
                """tanh(s_r) -> V; w = pri*V; b = sum_d w; softmax prep."""
                P = _pairs(mod)
                t_tmp = sm.tile([64, FN], F32, tag=f"vt_{mod}", name=f"vt_{mod}")
                scale = (1.0 / NV) if r == 0 else 1.0
                nc.scalar.activation(t_tmp[:], s_glob[mod][:], AF.Tanh, scale=scale)
                if r == 0:
                    nc.vector.tensor_copy(Vf[mod][:], t_tmp[:])
                else:
                    nc.vector.tensor_tensor(
                        out=Vf[mod][:], in0=Vf[mod][:], in1=t_tmp[:], op=ALU.add
                    )
                nc.vector.tensor_copy(vvbf[mod][0:64, :], Vf[mod][:])
                nc.vector.tensor_copy(vvbf[mod][64:128, :], Vf[mod][:])
                for p in range(P):
                    w_t = wk.tile([128, FN], BF16, tag="w", name="w_t")
                    nc.vector.tensor_tensor(
                        out=w_t[:], in0=pri[mod][p][:], in1=vvbf[mod][:], op=ALU.mult
                    )
                    nc.vector.tensor_reduce(
                        out=bstate[mod][:, p * NV : (p + 1) * NV],
                        in_=w_t.rearrange("q (d n) -> q n d", d=DC),
                        axis=mybir.AxisListType.X,
                        op=ALU.add,
                    )
                nc.scalar.activation(estate[mod][:], bstate[mod][:], AF.Exp)
                nc.vector.tensor_reduce(
                    out=den[mod][:],
                    in_=estate[mod].rearrange("q (t n) -> q t n", n=NV),
                    axis=mybir.AxisListType.X,
                    op=ALU.add,
                )
                nc.vector.reciprocal(rinv[mod][:], den[mod][:])
                nc.vector.tensor_tensor(
                    out=rcbf[mod].rearrange("q (t n) -> q t n", n=NV),
                    in0=estate[mod].rearrange("q (t n) -> q t n", n=NV),
                    in1=rinv[mod].unsqueeze(2).broadcast_to([128, P, NV]),
                    op=ALU.mult,
                )

            def mul1_and_s(mod, r):
                """m = rc * pri ; s_psum = sum_t m via selector matmuls."""
                P = _pairs(mod)
                s_ps = ps_s.tile([64, FN], F32, tag=f"s_{mod}", name=f"s_{mod}")
                for p in range(P):
                    m_t = wk.tile([128, FN], BF16, tag="m", name="m_t")
                    nc.vector.tensor_tensor(
                        out=m_t.rearrange("q (d n) -> q d n", d=DC),
                        in0=pri[mod][p].rearrange("q (d n) -> q d n", d=DC),
                        in1=rcbf[mod][:, p * NV : (p + 1) * NV]
                        .unsqueeze(1)
                        .broadcast_to([128, DC, NV]),
                        op=ALU.mult,
                    )
                    nc.tensor.matmul(
                        s_ps[:],
                        sel[:],
                        m_t[:],
                        start=(p == 0),
                        stop=(p == P - 1),
                    )
                return s_ps

            for r in range(ROUNDS):
                for mod in ORDER:
                    v_update_and_b(mod, r)
                    s_ps = mul1_and_s(mod, r + 1)
                    if r < ROUNDS - 1:
                        emit_allreduce(mod, r + 1, s_ps)
                    else:
                        s_out = sm.tile([64, FN], F32, tag=f"so_{mod}", name=f"so_{mod}")
                        nc.scalar.copy(s_out[:], s_ps[:])
                        nc.sync.dma_start(out_d[mod][:], s_out[:])

    nc.compile()
    return nc


def _host_prep(inputs):
    """Build the 8 per-core input maps (T-sharded, PE-ready layouts)."""
    sel = np.concatenate([np.eye(64, dtype=np.float32)] * 2, axis=0).astype(
        ml_dtypes.bfloat16
    )
    in_maps = []
    for c in range(N_CORES):
        m = {"sel": sel}
        for mod in ORDER:
            T = T_DIMS[mod]
            Tc = T // N_CORES
            P = Tc // 2
            t0 = c * Tc
            x = np.asarray(inputs[mod], dtype=np.float32)  # [B, T, J]
            W = np.asarray(inputs[W_NAMES[mod]], dtype=np.float32)  # [T,NV,J,DC]
            xs = np.ascontiguousarray(
                x[:, t0 : t0 + Tc, :].transpose(1, 2, 0)
            )  # [Tc, J, B]
            xb = np.zeros((P, 128, 128), dtype=np.float32)
            xb[:, 0:64, 0:64] = xs[0::2]
            xb[:, 64:128, 64:128] = xs[1::2]
            wt = W[t0 : t0 + Tc].transpose(0, 2, 3, 1).reshape(Tc, J, FN)
            # wt[t, j, d*32+n] = W[t, n, j, d]
            wr = np.empty((P, 128, FN), dtype=np.float32)
            wr[:, 0:64, :] = wt[0::2]
            wr[:, 64:128, :] = wt[1::2]
            m[f"xb_{mod}"] = xb
            m[f"wr_{mod}"] = np.ascontiguousarray(wr)
        in_maps.append(m)
    return in_maps


def _gather(results):
    outs = []
    for mod in OUT_ORDER:
        s = np.zeros((B, FN), dtype=np.float64)
        for c in range(N_CORES):
            s += np.asarray(results[c][f"out_{mod}"], dtype=np.float64)
        o = np.tanh(s.astype(np.float32))
        outs.append(np.ascontiguousarray(o.reshape(B, DC, NV).transpose(0, 2, 1)))
    return tuple(outs)


def kernel(**inputs):
    if "nc" not in _CACHE:
        _CACHE["nc"] = _build()
    nc = _CACHE["nc"]
    in_maps = _host_prep(inputs)
    res = run_bass_kernel_spmd(nc, in_maps, core_ids=list(range(N_CORES)))
    return _gather(res.results)
